# revision 21
# baseline (speedup 1.0000x reference)
"""Trainium2 Bass kernel for nn_ModelSimplest_11596411699489 (v4, fp8 DoubleRow).

Model: 4D conv (valid, 13^4 kernel, 1->3 ch, 18^4 -> 6^4) + bias + relu
       -> flatten (3888) -> dense (3888->2) + bias -> softmax.  B=512.

The conv is lowered to matmuls over (z,w)-plane Toeplitz blocks: for each
output block (ox,oy) and kernel-plane offset (kx,ky), the input plane
(ox+kx, oy+ky) contributes to the 108 outputs (co,oz,ow) of the block via a
[324 x 108] Toeplitz matrix, accumulated in PSUM over the 169 (kx,ky) taps.

fp8 (e4m3) inputs/weights with DoubleRow matmuls: each matmul contracts 256
plane rows ([128 partitions x 2 slots]).  Per kx: 13 DR-256 main chunks
(plane rows 0..255 per ky) plus the 13 taps' 68-row leftovers packed
densely into 4 more DR chunks (884 rows -> 3x256 + 116, crossing tap
boundaries) = 17 weight chunks, each shared by 6 matmuls (3 block-rows x
pair/single).  Adjacent oy blocks pair into one N=512 matmul (one PSUM
bank); oy=2 is a single N=256.  M is padded 108->112 to keep the DoubleRow
weight AP 16-byte aligned.  x is scaled by 2^4 and conv weights by 2^12
before the fp8 cast (both near-subnormal otherwise); the 2^-16 descale
folds into the evacuation activation's scale.  Feats/dense in bf16;
softmax fp32.

Sharding (8 cores): output (ox,oy) 6x6 grid split into 4 quadrants of 3x3
blocks; batch split in half.  core = 4*h + q.  Each core computes feats for
its 9 blocks / 256 samples and partial dense logits; an AllReduce over the
4 cores sharing a batch half combines them, then softmax (replicated).
Host concatenates the batch halves from cores 0 and 4.  (KERNEL_COLL=rs
uses ReduceScatter + per-core 64-sample softmax instead; =ag uses
AllGather + on-device sum.)
"""

import os
import sys

if "/opt/trn_rl_repo" not in sys.path:
    sys.path.insert(0, "/opt/trn_rl_repo")

import numpy as np
import ml_dtypes

F8 = ml_dtypes.float8_e4m3
BF16 = ml_dtypes.bfloat16

B, S, KS, SO, COUT = 512, 18, 13, 6, 3
PLANE = S * S            # 324
NB = B // 2              # batch per core (half)
M = COUT * SO * SO       # 108 outputs per block (co,oz,ow)
MP = 112                 # M padded for 16B-aligned DR weight AP
NBLK = 9                 # 3x3 blocks per quadrant
LEFT = PLANE - 256       # 68 leftover rows per tap
NSTK = 4                 # leftover chunks per kx (884 rows -> 3x256+116)
SX = 16.0                # x scale before fp8 cast
SW = 4096.0              # conv weight scale before fp8 cast
DESCALE = 1.0 / (SX * SW)
COLL = os.environ.get("KERNEL_COLL", "ar")

# leftover packing: row j of the concatenated per-tap leftovers lands in
# chunk j//256, partition (j%256)//2, slot j%2
_jj = np.arange(NSTK * 256)
_valid = _jj < KS * LEFT
_taps = np.where(_valid, _jj // LEFT, 0).reshape(NSTK, 128, 2)
_rows = np.where(_valid, _jj % LEFT, 0).reshape(NSTK, 128, 2)
_mask = _valid.reshape(NSTK, 128, 2)
# chunk c may touch planes tap..tap+2; schedule it after main tap maxtap(c)
_maxtap = _taps.reshape(NSTK, -1).max(axis=1)

_cache = {}


def _build_nc():
    import concourse.mybir as mybir
    import concourse.tile as tile
    from concourse import bacc

    f8 = mybir.dt.float8e4
    bf = mybir.dt.bfloat16
    f32 = mybir.dt.float32
    DR = mybir.MatmulPerfMode.DoubleRow

    nc = bacc.Bacc(num_devices=8)

    # x main plane rows 0..255: [X, part r, slot j, y, b] = x[X, y, 128j+r, b]
    xm_d = nc.dram_tensor("xm", [15, 128, 2, 15, NB], f8, kind="ExternalInput")
    # packed leftovers: [X, r, c, slot i, s, b]
    #   = x[X, taps[c,r,i]+s, 256+rows[c,r,i], b]
    xs_d = nc.dram_tensor("xs", [15, 128, NSTK, 2, 3, NB], f8, kind="ExternalInput")
    # weights (Toeplitz blocks, fp8, pre-scaled)
    wm_d = nc.dram_tensor("wm", [KS, 128, KS, 2, MP], f8, kind="ExternalInput")
    ws_d = nc.dram_tensor("ws", [KS, 128, NSTK, 2, MP], f8, kind="ExternalInput")
    cb_d = nc.dram_tensor("cb", [MP, 1], f32, kind="ExternalInput")
    wd_d = nc.dram_tensor("wd", [NBLK, MP, 2], bf, kind="ExternalInput")
    db_d = nc.dram_tensor("db", [128, 2], f32, kind="ExternalInput")
    out_rows = 64 if COLL == "rs" else NB
    out_d = nc.dram_tensor("out", [out_rows, 2], f32, kind="ExternalOutput")

    with tile.TileContext(nc) as tc:
        with (
            tc.tile_pool(name="xrows", bufs=1) as xpool,
            tc.tile_pool(name="wpool", bufs=1) as wpool,
            tc.tile_pool(name="feats", bufs=1) as fpool,
            tc.tile_pool(name="small", bufs=1) as spool,
            tc.tile_pool(name="psum", bufs=1, space="PSUM") as ppool,
            tc.tile_pool(name="dram", bufs=1, space="DRAM") as dpool,
        ):
            # accumulators: per block-row i, a y-pair (oy=0,1) -> [MP, 2*NB]
            # (one psum bank) and a single (oy=2) -> [MP, NB].
            pair_acc = [
                ppool.tile([MP, 2 * NB], f32, tag=f"pacc{i}", name=f"pacc{i}")
                for i in range(3)
            ]
            sing_acc = [
                ppool.tile([MP, NB], f32, tag=f"sacc{i}", name=f"sacc{i}")
                for i in range(3)
            ]

            xmt, xst = {}, {}
            qengs = [nc.sync, nc.scalar, nc.gpsimd]

            def alloc_xrow(X):
                m_t = xpool.tile([128, 2, 15, NB], f8, tag="xm", bufs=5)
                s_t = xpool.tile([128, NSTK, 2, 3, NB], f8, tag="xs", bufs=5)
                xmt[X], xst[X] = m_t, s_t
                return m_t, s_t

            def load_xrow(X):
                m_t, s_t = alloc_xrow(X)
                nc.sync.dma_start(out=m_t[:], in_=xm_d[X])
                nc.scalar.dma_start(out=s_t[:], in_=xs_d[X])

            wtiles = {}

            def load_wts(kx, first=False):
                wm_t = wpool.tile([128, KS, 2, MP], f8, tag="wm", bufs=3)
                ws_t = wpool.tile([128, NSTK, 2, MP], f8, tag="ws", bufs=3)
                if first:
                    # kx=0 weights off the queues carrying the first x pieces
                    nc.gpsimd.dma_start(out=wm_t[:], in_=wm_d[kx])
                    nc.gpsimd.dma_start(out=ws_t[:], in_=ws_d[kx])
                else:
                    qengs[kx % 3].dma_start(out=wm_t[:], in_=wm_d[kx])
                    qengs[(kx + 1) % 3].dma_start(out=ws_t[:], in_=ws_d[kx])
                wtiles[kx] = (wm_t, ws_t)

            # chunk schedule per kx: main taps with leftover chunk c placed
            # once its last tap's planes are in consumption order
            chunks = []
            nxt = 0
            for ky in range(KS):
                chunks.append(("m", ky))
                while nxt < NSTK and _maxtap[nxt] <= ky:
                    chunks.append(("s", nxt))
                    nxt += 1
            assert nxt == NSTK and len(chunks) == KS + NSTK

            # startup: hand-scheduled so the first chunk's operands (wm ky 0:4,
            # xm y 0:3 of X=0..2) land first, one queue per x-row
            wm0 = wpool.tile([128, KS, 2, MP], f8, tag="wm", bufs=3)
            ws0 = wpool.tile([128, NSTK, 2, MP], f8, tag="ws", bufs=3)
            wtiles[0] = (wm0, ws0)
            first3 = [alloc_xrow(X) for X in range(3)]
            nc.gpsimd.dma_start(out=wm0[:, 0:4], in_=wm_d[0][:, 0:4])
            for y0, y1 in ((0, 5), (5, 10), (10, 15)):
                for X in range(3):
                    qengs[X].dma_start(
                        out=first3[X][0][:, :, y0:y1, :], in_=xm_d[X, :, :, y0:y1]
                    )
                if y0 == 5:
                    nc.gpsimd.dma_start(out=wm0[:, 4:13], in_=wm_d[0][:, 4:13])
            nc.gpsimd.dma_start(out=ws0[:], in_=ws_d[0])
            for X in range(3):
                qengs[X].dma_start(out=first3[X][1][:, 0:2], in_=xs_d[X][:, 0:2])
            for X in range(3):
                qengs[X].dma_start(out=first3[X][1][:, 2:4], in_=xs_d[X][:, 2:4])

            for kx in range(KS):
                if kx + 1 < KS:
                    load_wts(kx + 1)
                if kx + 3 <= 14:
                    load_xrow(kx + 3)
                wm_t, ws_t = wtiles.pop(kx)
                for ci, (kind, idx) in enumerate(chunks):
                    first = kx == 0 and ci == 0
                    last = kx == KS - 1 and ci == len(chunks) - 1
                    if kind == "m":
                        lhsT = wm_t[:, idx, :, :]
                    else:
                        lhsT = ws_t[:, idx, :, :]
                    for i in range(3):
                        X = i + kx
                        if kind == "m":
                            rp = xmt[X][:, :, idx : idx + 2, :]
                            rs = xmt[X][:, :, idx + 2, :]
                        else:
                            rp = xst[X][:, idx, :, 0:2, :]
                            rs = xst[X][:, idx, :, 2, :]
                        nc.tensor.matmul(
                            pair_acc[i][:, :], lhsT=lhsT, rhs=rp,
                            start=first, stop=last, perf_mode=DR,
                        )
                        nc.tensor.matmul(
                            sing_acc[i][:, :], lhsT=lhsT, rhs=rs,
                            start=first, stop=last, perf_mode=DR,
                        )
                del xmt[kx], xst[kx]

            # constants issued program-late so their queue time lands while
            # gpsimd is idle mid-stream (only needed at evac)
            cb_t = spool.tile([MP, 1], f32, tag="cb")
            nc.gpsimd.dma_start(out=cb_t[:], in_=cb_d[:])
            db_t = spool.tile([128, 2], f32, tag="db")
            nc.gpsimd.dma_start(out=db_t[:], in_=db_d[:])
            wd_ts = []
            for bi in range(NBLK):
                t = spool.tile([MP, 2], bf, tag=f"wd{bi}")
                nc.gpsimd.dma_start(out=t[:], in_=wd_d[bi])
                wd_ts.append(t)

            # evac + relu + bias + descale; feats in bf16 for the dense
            feats = []
            for i in range(3):
                pf = fpool.tile([MP, 2 * NB], bf, tag=f"pfeat{i}", name=f"pf{i}")
                nc.scalar.activation(
                    pf[:],
                    pair_acc[i][:],
                    mybir.ActivationFunctionType.Relu,
                    bias=cb_t[:],
                    scale=DESCALE,
                )
                sf = fpool.tile([MP, NB], bf, tag=f"sfeat{i}", name=f"sf{i}")
                nc.scalar.activation(
                    sf[:],
                    sing_acc[i][:],
                    mybir.ActivationFunctionType.Relu,
                    bias=cb_t[:],
                    scale=DESCALE,
                )
                feats += [(pf, 0), (pf, NB), (sf, 0)]

            # dense partials: logits[b, cls] = sum_f feats[f, b] * wd[f, cls]
            cc_in = dpool.tile([4, 64, 2], f32, tag="ccin")
            for hh in range(2):
                dacc = ppool.tile([128, 2], f32, tag="dacc", bufs=2)
                for bi in range(NBLK):
                    ft, off = feats[bi]
                    nc.tensor.matmul(
                        dacc[:, :],
                        lhsT=ft[:, off + hh * 128 : off + (hh + 1) * 128],
                        rhs=wd_ts[bi][:],
                        start=(bi == 0),
                        stop=(bi == NBLK - 1),
                    )
                lg = spool.tile([128, 2], f32, tag=f"lg{hh}")
                nc.vector.tensor_copy(lg[:], dacc[:])
                nc.sync.dma_start(out=cc_in[2 * hh : 2 * hh + 2], in_=lg[:])

            groups = [[0, 1, 2, 3], [4, 5, 6, 7]]
            if COLL == "rs":
                cc_out = dpool.tile([64, 2], f32, tag="ccout")
                nc.gpsimd.collective_compute(
                    "ReduceScatter", mybir.AluOpType.add, replica_groups=groups,
                    ins=[cc_in.opt()], outs=[cc_out.opt()],
                )
                lr = spool.tile([64, 2], f32, tag="lr")
                nc.sync.dma_start(out=lr[:], in_=cc_out[:])
                lbs = [(lr, 64, 0)]
            elif COLL == "ag":
                cc_out = dpool.tile([4, 4, 64, 2], f32, tag="ccout")
                nc.gpsimd.collective_compute(
                    "AllGather", mybir.AluOpType.bypass, replica_groups=groups,
                    ins=[cc_in.opt()], outs=[cc_out.opt()],
                )
                lbs = []
                for hh in range(2):
                    gs = []
                    for g in range(4):
                        gt = spool.tile([128, 2], f32, tag=f"g{hh}_{g}")
                        nc.sync.dma_start(
                            out=gt[:], in_=cc_out[g, 2 * hh : 2 * hh + 2]
                        )
                        gs.append(gt)
                    a0 = spool.tile([128, 2], f32, tag=f"a0_{hh}")
                    nc.vector.tensor_add(a0[:], gs[0][:], gs[1][:])
                    a1 = spool.tile([128, 2], f32, tag=f"a1_{hh}")
                    nc.vector.tensor_add(a1[:], gs[2][:], gs[3][:])
                    a2 = spool.tile([128, 2], f32, tag=f"a2_{hh}")
                    nc.vector.tensor_add(a2[:], a0[:], a1[:])
                    lbs.append((a2, 128, hh * 128))
            else:
                cc_out = dpool.tile([4, 64, 2], f32, tag="ccout")
                nc.gpsimd.collective_compute(
                    "AllReduce", mybir.AluOpType.add, replica_groups=groups,
                    ins=[cc_in.opt()], outs=[cc_out.opt()],
                )
                lbs = []
                for hh in range(2):
                    lr = spool.tile([128, 2], f32, tag=f"lr{hh}")
                    nc.sync.dma_start(
                        out=lr[:], in_=cc_out[2 * hh : 2 * hh + 2]
                    )
                    lbs.append((lr, 128, hh * 128))

            for n, (lr, rows, o0) in enumerate(lbs):
                lb = spool.tile([rows, 2], f32, tag=f"lb{n}")
                nc.vector.tensor_add(lb[:], lr[:rows, :], db_t[:rows, :])
                ex = spool.tile([rows, 2], f32, tag=f"ex{n}")
                nc.scalar.activation(ex[:], lb[:], mybir.ActivationFunctionType.Exp)
                sm = spool.tile([rows, 1], f32, tag=f"sm{n}")
                nc.vector.reduce_sum(sm[:], ex[:], axis=mybir.AxisListType.X)
                rc = spool.tile([rows, 1], f32, tag=f"rc{n}")
                nc.vector.reciprocal(rc[:], sm[:])
                pr = spool.tile([rows, 2], f32, tag=f"pr{n}")
                nc.vector.tensor_scalar_mul(pr[:], ex[:], rc[:])
                nc.sync.dma_start(out=out_d[o0 : o0 + rows, :], in_=pr[:])

    nc.finalize()
    return nc


def _quant8(a):
    return np.clip(a, -240.0, 240.0).astype(F8)


def _build_w(conv_w):
    """conv_w [3,1,13,13,13,13] -> (wm, ws) fp8 Toeplitz chunk tiles."""
    s = np.arange(PLANE)
    z, w_ = s // S, s % S
    m = np.arange(M)
    co = m // (SO * SO)
    oz = (m % (SO * SO)) // SO
    ow = m % SO
    dz = z[:, None] - oz[None, :]                # [324,108]
    dw = w_[:, None] - ow[None, :]
    valid = (dz >= 0) & (dz < KS) & (dw >= 0) & (dw < KS)
    dzc = np.clip(dz, 0, KS - 1)
    dwc = np.clip(dw, 0, KS - 1)
    cw = conv_w[:, 0] * SW                       # [3,13,13,13,13] scaled
    cob = np.broadcast_to(co[None, :], dz.shape)

    W = np.zeros((KS, KS, PLANE, MP), np.float32)
    for kx in range(KS):
        for ky in range(KS):
            vals = cw[cob, kx, ky, dzc, dwc]     # [324,108]
            W[kx, ky, :, :M] = np.where(valid, vals, 0.0)

    wm = np.zeros((KS, 128, KS, 2, MP), np.float32)
    for ky in range(KS):
        for j in range(2):
            wm[:, :, ky, j, :] = W[:, ky, 128 * j : 128 * (j + 1), :]
    # packed leftovers: ws[kx, r, c, i, :] = W[kx, taps[c,r,i], 256+rows[c,r,i]]
    ws = W[:, _taps, 256 + _rows, :] * _mask[None, :, :, :, None]
    ws = np.ascontiguousarray(ws.transpose(0, 2, 1, 3, 4))  # [KS,128,NSTK,2,MP]
    return _quant8(wm), _quant8(ws)


def _build_inputs(x, conv_w, conv_b, dense_w, dense_b):
    x6 = np.ascontiguousarray(x.reshape(B, S, S, PLANE))
    wm, ws = _build_w(conv_w)

    m = np.arange(M)
    co = m // (SO * SO)
    oz = (m % (SO * SO)) // SO
    ow = m % SO

    cb = np.zeros((MP, 1), np.float32)
    cb[:M, 0] = conv_b[co]
    db = np.tile(dense_b[None, :].astype(np.float32), (128, 1))

    in_maps = []
    for core in range(8):
        q, h = core % 4, core // 4
        qx0, qy0 = 3 * (q // 2), 3 * (q % 2)
        slab = x6[h * NB : (h + 1) * NB, qx0 : qx0 + 15, qy0 : qy0 + 15, :]
        t = _quant8(np.transpose(slab, (1, 2, 3, 0)) * SX)  # [X, y, s, b] fp8
        # main: [15, 128, 2, 15, NB] = t[X, y, 128j+r, b] -> (X, r, j, y, b)
        xm = np.ascontiguousarray(
            t[:, :, :256, :]
            .reshape(15, 15, 2, 128, NB)
            .transpose(0, 3, 2, 1, 4)
        )
        xs = np.empty((15, 128, NSTK, 2, 3, NB), F8)
        for s_ in range(3):
            g = t[:, _taps + s_, 256 + _rows, :]  # [15, NSTK, 128, 2, NB]
            xs[:, :, :, :, s_, :] = g.transpose(0, 2, 1, 3, 4)

        wd = np.zeros((NBLK, MP, 2), BF16)
        for bi in range(NBLK):
            ox, oy = qx0 + bi // 3, qy0 + bi % 3
            f = co * (SO**4) + ox * (SO**3) + oy * (SO**2) + oz * SO + ow
            wd[bi, :M, :] = dense_w[:, f].T.astype(BF16)
        in_maps.append(
            {"xm": xm, "xs": xs, "wm": wm, "ws": ws, "cb": cb, "wd": wd, "db": db}
        )
    return in_maps


def _run(in_maps, trace=False):
    from concourse.bass_utils import run_bass_kernel_spmd

    if "nc" not in _cache:
        _cache["nc"] = _build_nc()
    return run_bass_kernel_spmd(_cache["nc"], in_maps, list(range(8)), trace=trace)


def kernel(x, conv_w, conv_b, dense_w, dense_b, _trace=False):
    x = np.asarray(x, np.float32)
    conv_w = np.asarray(conv_w, np.float32)
    conv_b = np.asarray(conv_b, np.float32)
    dense_w = np.asarray(dense_w, np.float32)
    dense_b = np.asarray(dense_b, np.float32)

    in_maps = _build_inputs(x, conv_w, conv_b, dense_w, dense_b)
    res = _run(in_maps, trace=_trace)
    if COLL == "rs":
        # core 4h+q holds the softmax for samples [256h + 64q, 256h + 64(q+1))
        out = np.concatenate([res.results[c]["out"] for c in range(8)], axis=0)
    else:
        out = np.concatenate(
            [res.results[0]["out"], res.results[4]["out"]], axis=0
        )
    if _trace:
        return out, res
    return out


# revision 22
# speedup vs baseline: 1.0282x; 1.0282x over previous
"""Trainium2 Bass kernel for nn_ModelSimplest_11596411699489 (v4, fp8 DoubleRow).

Model: 4D conv (valid, 13^4 kernel, 1->3 ch, 18^4 -> 6^4) + bias + relu
       -> flatten (3888) -> dense (3888->2) + bias -> softmax.  B=512.

The conv is lowered to matmuls over (z,w)-plane Toeplitz blocks: for each
output block (ox,oy) and kernel-plane offset (kx,ky), the input plane
(ox+kx, oy+ky) contributes to the 108 outputs (co,oz,ow) of the block via a
[324 x 108] Toeplitz matrix, accumulated in PSUM over the 169 (kx,ky) taps.

fp8 (e4m3) inputs/weights with DoubleRow matmuls: each matmul contracts 256
plane rows ([128 partitions x 2 slots]).  Per kx: 13 DR-256 main chunks
(plane rows 0..255 per ky) plus the 13 taps' 68-row leftovers packed
densely into 4 more DR chunks (884 rows -> 3x256 + 116, crossing tap
boundaries) = 17 weight chunks, each shared by 6 matmuls (3 block-rows x
pair/single).  Adjacent oy blocks pair into one N=512 matmul (one PSUM
bank); oy=2 is a single N=256.  M is padded 108->112 to keep the DoubleRow
weight AP 16-byte aligned.  x is scaled by 2^4 and conv weights by 2^12
before the fp8 cast (both near-subnormal otherwise); the 2^-16 descale
folds into the evacuation activation's scale.  Feats/dense in bf16;
softmax fp32.

Sharding (8 cores): output (ox,oy) 6x6 grid split into 4 quadrants of 3x3
blocks; batch split in half.  core = 4*h + q.  Each core computes feats for
its 9 blocks / 256 samples and partial dense logits; an AllReduce over the
4 cores sharing a batch half combines them, then softmax (replicated).
Host concatenates the batch halves from cores 0 and 4.  (KERNEL_COLL=rs
uses ReduceScatter + per-core 64-sample softmax instead; =ag uses
AllGather + on-device sum.)
"""

import os
import sys

if "/opt/trn_rl_repo" not in sys.path:
    sys.path.insert(0, "/opt/trn_rl_repo")

import numpy as np
import ml_dtypes

F8 = ml_dtypes.float8_e4m3
BF16 = ml_dtypes.bfloat16

B, S, KS, SO, COUT = 512, 18, 13, 6, 3
PLANE = S * S            # 324
NB = B // 2              # batch per core (half)
M = COUT * SO * SO       # 108 outputs per block (co,oz,ow)
MP = 112                 # M padded for 16B-aligned DR weight AP
NBLK = 9                 # 3x3 blocks per quadrant
LEFT = PLANE - 256       # 68 leftover rows per tap
NSTK = 4                 # leftover chunks per kx (884 rows -> 3x256+116)
SX = 16.0                # x scale before fp8 cast
SW = 4096.0              # conv weight scale before fp8 cast
DESCALE = 1.0 / (SX * SW)
COLL = os.environ.get("KERNEL_COLL", "ar")

# leftover packing: row j of the concatenated per-tap leftovers lands in
# chunk j//256, partition (j%256)//2, slot j%2
_jj = np.arange(NSTK * 256)
_valid = _jj < KS * LEFT
_taps = np.where(_valid, _jj // LEFT, 0).reshape(NSTK, 128, 2)
_rows = np.where(_valid, _jj % LEFT, 0).reshape(NSTK, 128, 2)
_mask = _valid.reshape(NSTK, 128, 2)
# chunk c may touch planes tap..tap+2; schedule it after main tap maxtap(c)
_maxtap = _taps.reshape(NSTK, -1).max(axis=1)

_cache = {}


def _build_nc():
    import concourse.mybir as mybir
    import concourse.tile as tile
    from concourse import bacc

    f8 = mybir.dt.float8e4
    bf = mybir.dt.bfloat16
    f32 = mybir.dt.float32
    DR = mybir.MatmulPerfMode.DoubleRow

    nc = bacc.Bacc(num_devices=8)

    # x main plane rows 0..255: [X, part r, slot j, y, b] = x[X, y, 128j+r, b]
    xm_d = nc.dram_tensor("xm", [15, 128, 2, 15, NB], f8, kind="ExternalInput")
    # packed leftovers: [X, r, c, slot i, s, b]
    #   = x[X, taps[c,r,i]+s, 256+rows[c,r,i], b]
    xs_d = nc.dram_tensor("xs", [15, 128, NSTK, 2, 3, NB], f8, kind="ExternalInput")
    # weights (Toeplitz blocks, fp8, pre-scaled)
    wm_d = nc.dram_tensor("wm", [KS, 128, KS, 2, MP], f8, kind="ExternalInput")
    ws_d = nc.dram_tensor("ws", [KS, 128, NSTK, 2, MP], f8, kind="ExternalInput")
    cb_d = nc.dram_tensor("cb", [MP, 1], f32, kind="ExternalInput")
    wd_d = nc.dram_tensor("wd", [NBLK, MP, 2], bf, kind="ExternalInput")
    db_d = nc.dram_tensor("db", [128, 2], f32, kind="ExternalInput")
    out_rows = 64 if COLL == "rs" else NB
    out_d = nc.dram_tensor("out", [out_rows, 2], f32, kind="ExternalOutput")

    with tile.TileContext(nc) as tc:
        with (
            tc.tile_pool(name="xrows", bufs=1) as xpool,
            tc.tile_pool(name="wpool", bufs=1) as wpool,
            tc.tile_pool(name="feats", bufs=1) as fpool,
            tc.tile_pool(name="small", bufs=1) as spool,
            tc.tile_pool(name="psum", bufs=1, space="PSUM") as ppool,
            tc.tile_pool(name="dram", bufs=1, space="DRAM") as dpool,
        ):
            # accumulators: per block-row i, a y-pair (oy=0,1) -> [MP, 2*NB]
            # (one psum bank) and a single (oy=2) -> [MP, NB].
            pair_acc = [
                ppool.tile([MP, 2 * NB], f32, tag=f"pacc{i}", name=f"pacc{i}")
                for i in range(3)
            ]
            sing_acc = [
                ppool.tile([MP, NB], f32, tag=f"sacc{i}", name=f"sacc{i}")
                for i in range(3)
            ]

            xmt, xst = {}, {}
            qengs = [nc.sync, nc.scalar, nc.gpsimd]

            def alloc_xrow(X):
                m_t = xpool.tile([128, 2, 15, NB], f8, tag="xm", bufs=5)
                s_t = xpool.tile([128, NSTK, 2, 3, NB], f8, tag="xs", bufs=5)
                xmt[X], xst[X] = m_t, s_t
                return m_t, s_t

            def load_xrow(X):
                m_t, s_t = alloc_xrow(X)
                nc.sync.dma_start(out=m_t[:], in_=xm_d[X])
                nc.scalar.dma_start(out=s_t[:], in_=xs_d[X])

            wtiles = {}

            def load_wts(kx, first=False):
                wm_t = wpool.tile([128, KS, 2, MP], f8, tag="wm", bufs=3)
                ws_t = wpool.tile([128, NSTK, 2, MP], f8, tag="ws", bufs=3)
                if first:
                    # kx=0 weights off the queues carrying the first x pieces
                    nc.gpsimd.dma_start(out=wm_t[:], in_=wm_d[kx])
                    nc.gpsimd.dma_start(out=ws_t[:], in_=ws_d[kx])
                else:
                    qengs[kx % 3].dma_start(out=wm_t[:], in_=wm_d[kx])
                    qengs[(kx + 1) % 3].dma_start(out=ws_t[:], in_=ws_d[kx])
                wtiles[kx] = (wm_t, ws_t)

            # chunk schedule per kx: main taps with leftover chunk c placed
            # once its last tap's planes are in consumption order
            chunks = []
            nxt = 0
            for ky in range(KS):
                chunks.append(("m", ky))
                while nxt < NSTK and _maxtap[nxt] <= ky:
                    chunks.append(("s", nxt))
                    nxt += 1
            assert nxt == NSTK and len(chunks) == KS + NSTK

            # startup: hand-scheduled so the first chunk's operands (wm ky 0:4,
            # xm y 0:3 of X=0..2) land first, one queue per x-row
            wm0 = wpool.tile([128, KS, 2, MP], f8, tag="wm", bufs=3)
            ws0 = wpool.tile([128, NSTK, 2, MP], f8, tag="ws", bufs=3)
            wtiles[0] = (wm0, ws0)
            first3 = [alloc_xrow(X) for X in range(3)]
            nc.gpsimd.dma_start(out=wm0[:, 0:4], in_=wm_d[0][:, 0:4])
            for y0, y1 in ((0, 4), (4, 9), (9, 15)):
                for X in range(3):
                    qengs[X].dma_start(
                        out=first3[X][0][:, :, y0:y1, :], in_=xm_d[X, :, :, y0:y1]
                    )
                if y0 == 4:
                    nc.gpsimd.dma_start(out=wm0[:, 4:13], in_=wm_d[0][:, 4:13])
            nc.gpsimd.dma_start(out=ws0[:], in_=ws_d[0])
            for X in range(3):
                qengs[X].dma_start(out=first3[X][1][:, 0:2], in_=xs_d[X][:, 0:2])
            for X in range(3):
                qengs[X].dma_start(out=first3[X][1][:, 2:4], in_=xs_d[X][:, 2:4])

            for kx in range(KS):
                if kx + 1 < KS:
                    load_wts(kx + 1)
                if kx + 3 <= 14:
                    load_xrow(kx + 3)
                wm_t, ws_t = wtiles.pop(kx)
                for ci, (kind, idx) in enumerate(chunks):
                    first = kx == 0 and ci == 0
                    last = kx == KS - 1 and ci == len(chunks) - 1
                    if kind == "m":
                        lhsT = wm_t[:, idx, :, :]
                    else:
                        lhsT = ws_t[:, idx, :, :]
                    for i in range(3):
                        X = i + kx
                        if kind == "m":
                            rp = xmt[X][:, :, idx : idx + 2, :]
                            rs = xmt[X][:, :, idx + 2, :]
                        else:
                            rp = xst[X][:, idx, :, 0:2, :]
                            rs = xst[X][:, idx, :, 2, :]
                        nc.tensor.matmul(
                            pair_acc[i][:, :], lhsT=lhsT, rhs=rp,
                            start=first, stop=last, perf_mode=DR,
                        )
                        nc.tensor.matmul(
                            sing_acc[i][:, :], lhsT=lhsT, rhs=rs,
                            start=first, stop=last, perf_mode=DR,
                        )
                del xmt[kx], xst[kx]

            # constants issued program-late so their queue time lands while
            # gpsimd is idle mid-stream (only needed at evac)
            cb_t = spool.tile([MP, 1], f32, tag="cb")
            nc.gpsimd.dma_start(out=cb_t[:], in_=cb_d[:])
            db_t = spool.tile([128, 2], f32, tag="db")
            nc.gpsimd.dma_start(out=db_t[:], in_=db_d[:])
            wd_ts = []
            for bi in range(NBLK):
                t = spool.tile([MP, 2], bf, tag=f"wd{bi}")
                nc.gpsimd.dma_start(out=t[:], in_=wd_d[bi])
                wd_ts.append(t)

            # evac + relu + bias + descale; feats in bf16 for the dense
            feats = []
            for i in range(3):
                pf = fpool.tile([MP, 2 * NB], bf, tag=f"pfeat{i}", name=f"pf{i}")
                nc.scalar.activation(
                    pf[:],
                    pair_acc[i][:],
                    mybir.ActivationFunctionType.Relu,
                    bias=cb_t[:],
                    scale=DESCALE,
                )
                sf = fpool.tile([MP, NB], bf, tag=f"sfeat{i}", name=f"sf{i}")
                nc.scalar.activation(
                    sf[:],
                    sing_acc[i][:],
                    mybir.ActivationFunctionType.Relu,
                    bias=cb_t[:],
                    scale=DESCALE,
                )
                feats += [(pf, 0), (pf, NB), (sf, 0)]

            # dense partials: logits[b, cls] = sum_f feats[f, b] * wd[f, cls]
            cc_in = dpool.tile([4, 64, 2], f32, tag="ccin")
            for hh in range(2):
                dacc = ppool.tile([128, 2], f32, tag="dacc", bufs=2)
                for bi in range(NBLK):
                    ft, off = feats[bi]
                    nc.tensor.matmul(
                        dacc[:, :],
                        lhsT=ft[:, off + hh * 128 : off + (hh + 1) * 128],
                        rhs=wd_ts[bi][:],
                        start=(bi == 0),
                        stop=(bi == NBLK - 1),
                    )
                lg = spool.tile([128, 2], f32, tag=f"lg{hh}")
                nc.vector.tensor_copy(lg[:], dacc[:])
                nc.sync.dma_start(out=cc_in[2 * hh : 2 * hh + 2], in_=lg[:])

            groups = [[0, 1, 2, 3], [4, 5, 6, 7]]
            if COLL == "rs":
                cc_out = dpool.tile([64, 2], f32, tag="ccout")
                nc.gpsimd.collective_compute(
                    "ReduceScatter", mybir.AluOpType.add, replica_groups=groups,
                    ins=[cc_in.opt()], outs=[cc_out.opt()],
                )
                lr = spool.tile([64, 2], f32, tag="lr")
                nc.sync.dma_start(out=lr[:], in_=cc_out[:])
                lbs = [(lr, 64, 0)]
            elif COLL == "ag":
                cc_out = dpool.tile([4, 4, 64, 2], f32, tag="ccout")
                nc.gpsimd.collective_compute(
                    "AllGather", mybir.AluOpType.bypass, replica_groups=groups,
                    ins=[cc_in.opt()], outs=[cc_out.opt()],
                )
                lbs = []
                for hh in range(2):
                    gs = []
                    for g in range(4):
                        gt = spool.tile([128, 2], f32, tag=f"g{hh}_{g}")
                        nc.sync.dma_start(
                            out=gt[:], in_=cc_out[g, 2 * hh : 2 * hh + 2]
                        )
                        gs.append(gt)
                    a0 = spool.tile([128, 2], f32, tag=f"a0_{hh}")
                    nc.vector.tensor_add(a0[:], gs[0][:], gs[1][:])
                    a1 = spool.tile([128, 2], f32, tag=f"a1_{hh}")
                    nc.vector.tensor_add(a1[:], gs[2][:], gs[3][:])
                    a2 = spool.tile([128, 2], f32, tag=f"a2_{hh}")
                    nc.vector.tensor_add(a2[:], a0[:], a1[:])
                    lbs.append((a2, 128, hh * 128))
            else:
                cc_out = dpool.tile([4, 64, 2], f32, tag="ccout")
                nc.gpsimd.collective_compute(
                    "AllReduce", mybir.AluOpType.add, replica_groups=groups,
                    ins=[cc_in.opt()], outs=[cc_out.opt()],
                )
                lbs = []
                for hh in range(2):
                    lr = spool.tile([128, 2], f32, tag=f"lr{hh}")
                    nc.sync.dma_start(
                        out=lr[:], in_=cc_out[2 * hh : 2 * hh + 2]
                    )
                    lbs.append((lr, 128, hh * 128))

            for n, (lr, rows, o0) in enumerate(lbs):
                lb = spool.tile([rows, 2], f32, tag=f"lb{n}")
                nc.vector.tensor_add(lb[:], lr[:rows, :], db_t[:rows, :])
                ex = spool.tile([rows, 2], f32, tag=f"ex{n}")
                nc.scalar.activation(ex[:], lb[:], mybir.ActivationFunctionType.Exp)
                sm = spool.tile([rows, 1], f32, tag=f"sm{n}")
                nc.vector.reduce_sum(sm[:], ex[:], axis=mybir.AxisListType.X)
                rc = spool.tile([rows, 1], f32, tag=f"rc{n}")
                nc.vector.reciprocal(rc[:], sm[:])
                pr = spool.tile([rows, 2], f32, tag=f"pr{n}")
                nc.vector.tensor_scalar_mul(pr[:], ex[:], rc[:])
                nc.sync.dma_start(out=out_d[o0 : o0 + rows, :], in_=pr[:])

    nc.finalize()
    return nc


def _quant8(a):
    return np.clip(a, -240.0, 240.0).astype(F8)


def _build_w(conv_w):
    """conv_w [3,1,13,13,13,13] -> (wm, ws) fp8 Toeplitz chunk tiles."""
    s = np.arange(PLANE)
    z, w_ = s // S, s % S
    m = np.arange(M)
    co = m // (SO * SO)
    oz = (m % (SO * SO)) // SO
    ow = m % SO
    dz = z[:, None] - oz[None, :]                # [324,108]
    dw = w_[:, None] - ow[None, :]
    valid = (dz >= 0) & (dz < KS) & (dw >= 0) & (dw < KS)
    dzc = np.clip(dz, 0, KS - 1)
    dwc = np.clip(dw, 0, KS - 1)
    cw = conv_w[:, 0] * SW                       # [3,13,13,13,13] scaled
    cob = np.broadcast_to(co[None, :], dz.shape)

    W = np.zeros((KS, KS, PLANE, MP), np.float32)
    for kx in range(KS):
        for ky in range(KS):
            vals = cw[cob, kx, ky, dzc, dwc]     # [324,108]
            W[kx, ky, :, :M] = np.where(valid, vals, 0.0)

    wm = np.zeros((KS, 128, KS, 2, MP), np.float32)
    for ky in range(KS):
        for j in range(2):
            wm[:, :, ky, j, :] = W[:, ky, 128 * j : 128 * (j + 1), :]
    # packed leftovers: ws[kx, r, c, i, :] = W[kx, taps[c,r,i], 256+rows[c,r,i]]
    ws = W[:, _taps, 256 + _rows, :] * _mask[None, :, :, :, None]
    ws = np.ascontiguousarray(ws.transpose(0, 2, 1, 3, 4))  # [KS,128,NSTK,2,MP]
    return _quant8(wm), _quant8(ws)


def _build_inputs(x, conv_w, conv_b, dense_w, dense_b):
    x6 = np.ascontiguousarray(x.reshape(B, S, S, PLANE))
    wm, ws = _build_w(conv_w)

    m = np.arange(M)
    co = m // (SO * SO)
    oz = (m % (SO * SO)) // SO
    ow = m % SO

    cb = np.zeros((MP, 1), np.float32)
    cb[:M, 0] = conv_b[co]
    db = np.tile(dense_b[None, :].astype(np.float32), (128, 1))

    in_maps = []
    for core in range(8):
        q, h = core % 4, core // 4
        qx0, qy0 = 3 * (q // 2), 3 * (q % 2)
        slab = x6[h * NB : (h + 1) * NB, qx0 : qx0 + 15, qy0 : qy0 + 15, :]
        t = _quant8(np.transpose(slab, (1, 2, 3, 0)) * SX)  # [X, y, s, b] fp8
        # main: [15, 128, 2, 15, NB] = t[X, y, 128j+r, b] -> (X, r, j, y, b)
        xm = np.ascontiguousarray(
            t[:, :, :256, :]
            .reshape(15, 15, 2, 128, NB)
            .transpose(0, 3, 2, 1, 4)
        )
        xs = np.empty((15, 128, NSTK, 2, 3, NB), F8)
        for s_ in range(3):
            g = t[:, _taps + s_, 256 + _rows, :]  # [15, NSTK, 128, 2, NB]
            xs[:, :, :, :, s_, :] = g.transpose(0, 2, 1, 3, 4)

        wd = np.zeros((NBLK, MP, 2), BF16)
        for bi in range(NBLK):
            ox, oy = qx0 + bi // 3, qy0 + bi % 3
            f = co * (SO**4) + ox * (SO**3) + oy * (SO**2) + oz * SO + ow
            wd[bi, :M, :] = dense_w[:, f].T.astype(BF16)
        in_maps.append(
            {"xm": xm, "xs": xs, "wm": wm, "ws": ws, "cb": cb, "wd": wd, "db": db}
        )
    return in_maps


def _run(in_maps, trace=False):
    from concourse.bass_utils import run_bass_kernel_spmd

    if "nc" not in _cache:
        _cache["nc"] = _build_nc()
    return run_bass_kernel_spmd(_cache["nc"], in_maps, list(range(8)), trace=trace)


def kernel(x, conv_w, conv_b, dense_w, dense_b, _trace=False):
    x = np.asarray(x, np.float32)
    conv_w = np.asarray(conv_w, np.float32)
    conv_b = np.asarray(conv_b, np.float32)
    dense_w = np.asarray(dense_w, np.float32)
    dense_b = np.asarray(dense_b, np.float32)

    in_maps = _build_inputs(x, conv_w, conv_b, dense_w, dense_b)
    res = _run(in_maps, trace=_trace)
    if COLL == "rs":
        # core 4h+q holds the softmax for samples [256h + 64q, 256h + 64(q+1))
        out = np.concatenate([res.results[c]["out"] for c in range(8)], axis=0)
    else:
        out = np.concatenate(
            [res.results[0]["out"], res.results[4]["out"]], axis=0
        )
    if _trace:
        return out, res
    return out
